# revision 1
# baseline (speedup 1.0000x reference)
"""4-layer GCN (out = adj @ (h @ W) + b, stacked) on 8 trn2 NeuronCores.

Strategy (row-parallel over nodes):
  - Each core owns R = N/8 rows of adj (its output rows for every layer).
  - The PE contracts over the partition dim, so adj tiles must be fed
    k-major (transposed).  Pass 0 loads natural f32 adj tiles, casts to
    bf16 (DVE/ACT), transposes 128x128 tiles on the PE (identity matmul),
    uses them for the layer-0 GEMM, and stores the transposed bf16
    adjacency to a DRAM scratch.  Passes 1..3 stream that scratch at line
    rate (half the bytes of f32) with zero transpose work.
  - h^T shards are AllGather'd (bf16) between layers; Z = h @ W is
    computed redundantly per core (tiny).
  - All accumulation is fp32 in PSUM; only matmul operands are bf16.

kernel(**inputs) takes the full-size numpy inputs and returns the full
[N, 16] float32 output.
"""

import os

import numpy as np
import ml_dtypes

P = 128            # SBUF partitions / PE tile size
N_CORES = 8
SEG = 512          # fp32 PSUM bank width (free-dim elements)

# Full-problem config (must match the harness problem)
FULL_N = 16384
FULL_D_IN = 128
FULL_D_HID = 64
FULL_N_CLASSES = 16
FULL_N_HIDDEN_LAYERS = 2

_CACHE = {}
_LAST_RESULTS = None  # BassKernelResults of the most recent run (for test.py)


def _split_dma_waits(nc, mybir, max_waits=1, noop_waits=1):
    """Walrus' DMA pseudo-instruction supports at most 2 sem waits; Tile can
    emit 3+.  Hoist all waits of offending DMAs onto a NoOp on the issuing
    engine immediately before the DMA (same NX stream, so ordering holds)."""
    for f in nc.m.functions:
        for bb in f.blocks:
            insts = bb.instructions
            i = 0
            while i < len(insts):
                ins = insts[i]
                si = ins.sync_info
                if (
                    si is not None
                    and si.on_wait
                    and len(si.on_wait) > max_waits
                ):
                    waits = list(si.on_wait)
                    keep = waits[-max_waits:]
                    extra = waits[:-max_waits]
                    for j in range(0, len(extra), noop_waits):
                        noop = mybir.InstNoOp(
                            name=nc.get_next_instruction_name(),
                            engine=ins.engine,
                            ins=[],
                            outs=[],
                            sync_info=mybir.SyncInfo(
                                on_wait=extra[j : j + noop_waits], on_update=[]
                            ),
                        )
                        insts.insert(i, noop)
                        i += 1
                    ins.sync_info = mybir.SyncInfo(
                        on_wait=keep, on_update=list(si.on_update or [])
                    )
                i += 1


def _build(N, R, layer_dims):
    """Build the per-core Bass program.

    N: total nodes; R: rows per core; layer_dims: [(d_in, d_out), ...]
    """
    import concourse.bass as bass
    import concourse.mybir as mybir
    from concourse import tile, masks

    f32 = mybir.dt.float32
    bf16 = mybir.dt.bfloat16

    KB = N // P                    # contraction k-blocks
    TR = R // P                    # 128-col tiles per strip
    seg_w = min(SEG, R)            # psum segment width
    n_seg = R // seg_w
    tps = seg_w // P               # transpose tiles per segment
    n_layers = len(layer_dims)
    d_in0 = layer_dims[0][0]
    d_last = layer_dims[-1][1]

    nc = bass.Bass(trn_type="TRN2", num_devices=N_CORES)

    adj_d = nc.dram_tensor("adj_shard", [R, N], f32, kind="ExternalInput")
    xT_d = nc.dram_tensor("xT", [d_in0, N], bf16, kind="ExternalInput")
    w_d = [
        nc.dram_tensor(f"w{l}", [di, do], bf16, kind="ExternalInput")
        for l, (di, do) in enumerate(layer_dims)
    ]
    b_d = [
        nc.dram_tensor(f"b{l}", [do, 1], f32, kind="ExternalInput")
        for l, (di, do) in enumerate(layer_dims)
    ]
    outT_d = nc.dram_tensor("outT", [d_last, R], f32, kind="ExternalOutput")

    with tile.TileContext(nc) as tc:
        with (
            tc.tile_pool(name="const", bufs=1) as constp,
            tc.tile_pool(name="xt", bufs=1) as xtp,
            tc.tile_pool(name="z", bufs=2) as zp,
            tc.tile_pool(name="nat", bufs=2) as natp,
            tc.tile_pool(name="natb", bufs=3) as natbp,
            tc.tile_pool(name="strip", bufs=4) as stripp,
            tc.tile_pool(name="h", bufs=2) as hp,
            tc.tile_pool(name="hfull", bufs=1) as hfp,
            tc.tile_pool(name="pz", bufs=2, space="PSUM") as pzp,
            tc.tile_pool(name="pt", bufs=2, space="PSUM") as ptp,
            tc.tile_pool(name="ph", bufs=1, space="PSUM") as php,
            tc.tile_pool(name="dram", bufs=1, space="DRAM") as dramp,
        ):
            ident = constp.tile([P, P], bf16, tag="ident")
            masks.make_identity(nc, ident[:])

            w_sb, b_sb = [], []
            for l, (di, do) in enumerate(layer_dims):
                w = constp.tile([di, do], bf16, tag=f"w{l}")
                nc.sync.dma_start(w[:], w_d[l][:])
                b = constp.tile([do, 1], f32, tag=f"b{l}")
                nc.sync.dma_start(b[:], b_d[l][:])
                w_sb.append(w)
                b_sb.append(b)

            # x^T replicated; also serves as h0^T for the layer-0 Z stage.
            xt = xtp.tile([d_in0, N], bf16, tag="xt")
            nc.sync.dma_start(xt[:], xT_d[:])

            adjT = dramp.tile([N, R], bf16, tag="adjT")

            hT_bf = None  # gathered h^T [d, N] bf16 for layers >= 1
            for l in range(n_layers):
                di, do = layer_dims[l]
                last = l == n_layers - 1

                # ---- Z_l = h_l @ W_l, natural [k, do] layout, bf16 ----
                zbuf = zp.tile([P, KB * do], bf16, tag="zbuf")
                hsrc = xt if l == 0 else hT_bf
                for kb in range(KB):
                    pz = pzp.tile([P, do], f32, tag="pz")
                    nc.tensor.matmul(
                        pz[:],
                        hsrc[:, kb * P : (kb + 1) * P],
                        w_sb[l][:],
                        start=True,
                        stop=True,
                    )
                    nc.any.tensor_copy(zbuf[:, kb * do : (kb + 1) * do], pz[:])

                # ---- big GEMM: h_{l+1}^T[n, i] = sum_k Z[k, n] adjT[k, i] ----
                ph = php.tile([do, n_seg * seg_w], f32, tag="ph")
                KK = 2 if KB % 2 == 0 else 1  # k-blocks per natural load
                nat = None
                for kb in range(KB):
                    if l == 0:
                        # natural f32 chunk [128 i x KK*128 k] x TR tiles;
                        # KK=2 gives 1 KiB contiguous DMA lines (vs 512 B)
                        kk = kb % KK
                        if kk == 0:
                            nat = natp.tile([P, TR, KK, P], f32, tag="nat")
                            nc.sync.dma_start(
                                nat[:],
                                adj_d[:, kb * P : (kb + KK) * P].rearrange(
                                    "(t p) (kk k) -> p t kk k", p=P, k=P
                                ),
                            )
                        natb = natbp.tile([P, TR, P], bf16, tag="natb")
                        if TR > 1:
                            h1 = TR // 2
                            nc.any.tensor_copy(natb[:, :h1], nat[:, :h1, kk, :])
                            nc.any.tensor_copy(natb[:, h1:], nat[:, h1:, kk, :])
                        else:
                            nc.any.tensor_copy(natb[:], nat[:, :, kk, :])
                        strip = stripp.tile([P, R], bf16, tag="strip")
                        for s in range(n_seg):
                            pt = ptp.tile([P, seg_w], bf16, tag="pt")
                            for j in range(tps):
                                t = tps * s + j
                                nc.tensor.matmul(
                                    pt[:, j * P : (j + 1) * P],
                                    natb[:, t, :],
                                    ident[:],
                                    is_transpose=True,
                                    start=(j == 0),
                                    stop=(j == tps - 1),
                                )
                            nc.any.tensor_copy(
                                strip[:, s * seg_w : (s + 1) * seg_w], pt[:]
                            )
                        nc.sync.dma_start(adjT[kb * P : (kb + 1) * P, :], strip[:])
                    else:
                        strip = stripp.tile([P, R], bf16, tag="strip")
                        nc.sync.dma_start(
                            strip[:], adjT[kb * P : (kb + 1) * P, :]
                        )
                    for s in range(n_seg):
                        nc.tensor.matmul(
                            ph[:, s * seg_w : (s + 1) * seg_w],
                            zbuf[:, kb * do : (kb + 1) * do],
                            strip[:, s * seg_w : (s + 1) * seg_w],
                            start=(kb == 0),
                            stop=(kb == KB - 1),
                        )

                # ---- bias add (+ cast) and inter-layer AllGather ----
                if last:
                    hf = hp.tile([do, R], f32, tag="hf")
                    for s in range(n_seg):
                        nc.vector.tensor_scalar_add(
                            hf[:, s * seg_w : (s + 1) * seg_w],
                            ph[:, s * seg_w : (s + 1) * seg_w],
                            b_sb[l][:, 0:1],
                        )
                    nc.sync.dma_start(outT_d[:], hf[:])
                else:
                    hb = hp.tile([do, R], bf16, tag="hb")
                    for s in range(n_seg):
                        nc.vector.tensor_scalar_add(
                            hb[:, s * seg_w : (s + 1) * seg_w],
                            ph[:, s * seg_w : (s + 1) * seg_w],
                            b_sb[l][:, 0:1],
                        )
                    cc_in = dramp.tile([do, R], bf16, tag=f"ccin{l}")
                    nc.sync.dma_start(cc_in[:], hb[:])
                    cc_out = dramp.tile(
                        [N_CORES * do, R], bf16, addr_space="Shared", tag=f"ccout{l}"
                    )
                    nc.gpsimd.collective_compute(
                        "AllGather",
                        mybir.AluOpType.bypass,
                        replica_groups=[list(range(N_CORES))],
                        ins=[cc_in.opt()],
                        outs=[cc_out.opt()],
                    )
                    hT_bf = hfp.tile([do, N], bf16, tag="hfull")
                    nc.sync.dma_start(
                        hT_bf[:].rearrange("d (r i) -> d r i", i=R),
                        cc_out[:].rearrange("(r d) i -> d r i", d=do),
                    )
    _split_dma_waits(nc, mybir)
    return nc


def _prep_inputs(x, adj, W_in, b_in, W_hidden, b_hidden, W_out, b_out, N, R):
    bf = ml_dtypes.bfloat16
    xT = np.ascontiguousarray(np.asarray(x, dtype=np.float32).T).astype(bf)
    ws = [np.asarray(W_in)] + [np.asarray(W_hidden)[i] for i in range(np.asarray(W_hidden).shape[0])] + [np.asarray(W_out)]
    bs = [np.asarray(b_in)] + [np.asarray(b_hidden)[i] for i in range(np.asarray(b_hidden).shape[0])] + [np.asarray(b_out)]
    ws = [np.ascontiguousarray(w.astype(np.float32)).astype(bf) for w in ws]
    bs = [np.ascontiguousarray(b.astype(np.float32).reshape(-1, 1)) for b in bs]
    adj = np.asarray(adj, dtype=np.float32)
    in_maps = []
    for c in range(N_CORES):
        m = {"adj_shard": np.ascontiguousarray(adj[c * R : (c + 1) * R]), "xT": xT}
        for l, (w, b) in enumerate(zip(ws, bs)):
            m[f"w{l}"] = w
            m[f"b{l}"] = b
        in_maps.append(m)
    return in_maps


def _run(nc, in_maps, trace=False):
    from concourse.bass_utils import run_bass_kernel_spmd

    global _LAST_RESULTS
    try:
        res = run_bass_kernel_spmd(
            nc, in_maps, core_ids=list(range(N_CORES)), trace=trace
        )
    except ModuleNotFoundError:
        # NTFF profile hook unavailable in this container; rerun untraced.
        res = run_bass_kernel_spmd(
            nc, in_maps, core_ids=list(range(N_CORES)), trace=False
        )
    _LAST_RESULTS = res
    return res.results


def kernel(x, adj, W_in, b_in, W_hidden, b_hidden, W_out, b_out):
    N = FULL_N
    R = N // N_CORES
    layer_dims = (
        [(FULL_D_IN, FULL_D_HID)]
        + [(FULL_D_HID, FULL_D_HID)] * FULL_N_HIDDEN_LAYERS
        + [(FULL_D_HID, FULL_N_CLASSES)]
    )
    key = (N, R, tuple(layer_dims))
    if key not in _CACHE:
        _CACHE[key] = _build(N, R, layer_dims)
    nc = _CACHE[key]
    in_maps = _prep_inputs(
        x, adj, W_in, b_in, W_hidden, b_hidden, W_out, b_out, N, R
    )
    trace = os.environ.get("GCN_TRACE", "0") == "1"
    results = _run(nc, in_maps, trace=trace)
    out = np.empty((N, FULL_N_CLASSES), dtype=np.float32)
    for c in range(N_CORES):
        out[c * R : (c + 1) * R, :] = results[c]["outT"].T
    return out

